# revision 1
# baseline (speedup 1.0000x reference)
"""Cross-Spatial-Attention Trainium2 kernel (8 NeuronCores, spatial sharding).

Strategy: shard the 256-row image into 8 bands of 32 rows (both batch elements
on every core, 1-row halos for the 3x3 depthwise convs). All convolutions and
the attention application are then fully local; the only cross-core data is the
channel-gram / norm / mean statistics (one ~134KB AllReduce).

Key formulations:
  - dwconv3x3(conv1x1(x)) == sum over 9 taps of shifted matmuls with
    per-tap-combined weights (PSUM accumulation).
  - q,k are produced directly transposed ([n,c] layout) via
    out_chunk = x_chunk^T @ W_tap^T, so the channel gram needs no transpose
    pass and the spatial gate `sa` is a per-partition scalar.
  - softmax over a full 128x128 gram with a block-diagonal mask; the
    attention apply + output projection collapse into one matmul
    (Meff = proj @ attnBD), and the spectral gate folds into the
    projection weights for the dwconv(y) branch.
"""

import numpy as np
from contextlib import ExitStack

import concourse.bass as bass
import concourse.bacc as bacc
import concourse.tile as tile
from concourse import mybir
from concourse.bass_utils import run_bass_kernel_spmd

FP32 = mybir.dt.float32
BF16 = mybir.dt.bfloat16
AF = mybir.ActivationFunctionType
ALU = mybir.AluOpType

B, C, H, W = 2, 128, 256, 256
HD, DH = 8, 16
NCORES = 8
RPC = H // NCORES            # 32 rows per core
HH, WW = RPC + 2, W + 2      # 34 x 258 halo'd band
FREE = HH * WW               # 8772
NLOC = RPC * W               # 8192 output positions per band per batch
NCH_T = NLOC // 128          # 64 transposed chunks
NCH_A = NLOC // 512          # 16 layout-A chunks
NTOT = float(H * W)          # global spatial size


STAGE = 5
EXTRA_V = 0
EN_SA = True
EN_QK = True
EN_V = True
EN_NB = 2


def _emit(tc, io):
    nc = tc.nc
    ctx = ExitStack()

    wpool = ctx.enter_context(tc.tile_pool(name="wpool", bufs=1))
    xpool = ctx.enter_context(tc.tile_pool(name="xpool", bufs=2))
    ypool = ctx.enter_context(tc.tile_pool(name="ypool", bufs=2))
    vpool = ctx.enter_context(tc.tile_pool(name="vpool", bufs=2))
    y2pool = ctx.enter_context(tc.tile_pool(name="y2pool", bufs=2))
    spool = ctx.enter_context(tc.tile_pool(name="spool", bufs=2))
    rpool = ctx.enter_context(tc.tile_pool(name="rpool", bufs=8))
    mpool = ctx.enter_context(tc.tile_pool(name="mpool", bufs=2))
    opool = ctx.enter_context(tc.tile_pool(name="opool", bufs=3))
    # PSUM pools: psA(3) + psQK(3) + psG(2 tags x 1) = 8 banks
    psA = ctx.enter_context(tc.tile_pool(name="psA", bufs=2, space="PSUM"))
    psQK = ctx.enter_context(tc.tile_pool(name="psQK", bufs=3, space="PSUM"))
    psG = ctx.enter_context(tc.tile_pool(name="psG", bufs=1, space="PSUM"))
    dpool = ctx.enter_context(tc.tile_pool(name="dram", bufs=2, space="DRAM"))

    def dma(dst, src):
        nc.sync.dma_start(out=dst, in_=src)

    # ---- load weights/constants ----
    def wload(name, shape, dt=BF16):
        t = wpool.tile(shape, dt, tag=name)
        dma(t[:], io[name][:])
        return t

    w9qk = wload("w9qk", [128, 9 * 256])     # tap t at cols [256t:256t+256]
    w9v = wload("w9v", [128, 9 * 128])
    wyd = wload("wyd", [128, 9 * 128])
    saw1t = wload("saw1t", [128, 32])
    w2rep = wload("w2rep", [128, 32])
    w3rep = wload("w3rep", [128, 1])
    spw1t = wload("spw1t", [128, 16], FP32)
    spw2t = wload("spw2t", [16, 16], FP32)
    spw3t = wload("spw3t", [16, 128], FP32)
    projt = wload("projt", [128, 128], FP32)
    consts = wload("consts", [128, 386], FP32)
    eye = consts[:, 0:128]
    bdmask = consts[:, 128:256]
    tempp = consts[:, 256:257]
    onesrow = consts[0:1, 257:385]

    xts, yts, vts, y2ts, saTs = [], [], [], [], []
    arst = mpool.tile([128, 262], FP32, tag="arst")

    # ================= per-batch phase A =================
    for b in range(min(B, EN_NB)):
        cb = 131 * b
        xt = xpool.tile([128, FREE], BF16, tag="x")
        yt = ypool.tile([128, FREE], BF16, tag="y")
        dma(xt[:], io["xh"][b].rearrange("c h w -> c (h w)"))
        dma(yt[:], io["yh"][b].rearrange("c h w -> c (h w)"))
        xts.append(xt)
        yts.append(yt)

        en_sa = EN_SA if isinstance(EN_SA, bool) else EN_SA[b]
        en_qk = EN_QK if isinstance(EN_QK, bool) else EN_QK[b]
        en_v = EN_V if isinstance(EN_V, bool) else EN_V[b]
        if not en_sa:
            saT = mpool.tile([128, 64], FP32, tag="saT")
            nc.vector.memset(saT[:], 1.0)
            saTs.append(saT)
        # ---- spatial-attention gate: sa = sigmoid(w3 relu(w2 relu(w1 y))) ----
        # stage 1/2 outputs packed 4 chunks high per psum tile via col-tiling
        s1 = spool.tile([128, 2048], BF16, tag="s1")
        s2 = spool.tile([128, 2048], BF16, tag="s2")
        for g in range(4 if en_sa else 0):
            ps1 = psA.tile([128, 512], FP32, tag="a")
            for k in range(4):
                nn = 4 * g + k
                r0 = 2 * nn
                yv = yt[:].rearrange("p (h w) -> p h w", h=HH)[
                    :, r0 + 1:r0 + 3, 1:257]
                nc.tensor.matmul(ps1[32 * k:32 * k + 32, :], saw1t[:, :], yv,
                                 start=True, stop=True, tile_position=(0, 32 * k))
            eng = nc.vector if g % 2 == 0 else nc.scalar
            if g % 2 == 0:
                nc.vector.tensor_scalar_max(s1[:, 512 * g:512 * g + 512], ps1[:, :], 0.0)
            else:
                nc.scalar.activation(s1[:, 512 * g:512 * g + 512], ps1[:, :], AF.Relu)
        for g in range(4 if en_sa else 0):
            ps2 = psA.tile([128, 512], FP32, tag="a")
            for k in range(4):
                nc.tensor.matmul(ps2[32 * k:32 * k + 32, :],
                                 w2rep[32 * k:32 * k + 16, :],
                                 s1[32 * k:32 * k + 16, 512 * g:512 * g + 512],
                                 start=True, stop=True,
                                 tile_position=(32 * k, 32 * k))
            if g % 2 == 0:
                nc.vector.tensor_scalar_max(s2[:, 512 * g:512 * g + 512], ps2[:, :], 0.0)
            else:
                nc.scalar.activation(s2[:, 512 * g:512 * g + 512], ps2[:, :], AF.Relu)
        if en_sa:
            # stage 3: saT[n] packed as [128, 64] (col j holds n = 128j + p)
            saT_ps = psQK.tile([128, 64], FP32, tag="qk")
            for j in range(NCH_T):
                nn, off = j // 4, (j % 4) * 128
                g, k = nn // 4, nn % 4
                nc.tensor.matmul(saT_ps[:, j:j + 1],
                                 s2[32 * k:32 * k + 16,
                                    512 * g + off:512 * g + off + 128],
                                 w3rep[32 * k:32 * k + 16, :],
                                 start=True, stop=True, tile_position=(32 * k, 0))
            saT = mpool.tile([128, 64], FP32, tag="saT")
            nc.scalar.activation(saT[:], saT_ps[:], AF.Sigmoid)
            saTs.append(saT)
        saT = saTs[b]

        # ---- qk conv (transposed layout) + gram accumulation ----
        G = psG.tile([128, 256], FP32, tag="G")
        G2 = psG.tile([128, 128], FP32, tag="G2")
        if not en_qk:
            pz = psQK.tile([128, 256], FP32, tag="qk")
            rz = rpool.tile([128, 256], BF16, tag="ring")
            nc.tensor.matmul(pz[:, :], xt[:, 0:128], w9qk[:, 0:256],
                             start=True, stop=True)
            nc.vector.tensor_copy(rz[:], pz[:, :])
            nc.tensor.matmul(G[:, 0:256], rz[:, 0:128], rz[:, 0:256],
                             start=True, stop=True, skip_group_check=True)
            nc.tensor.matmul(G2[:, :], rz[:, 128:256], rz[:, 128:256],
                             start=True, stop=True, skip_group_check=True)
        for j in range(NCH_T if en_qk else 0):
            r, c0 = j // 2, (j % 2) * 128
            pqk = psQK.tile([128, 256], FP32, tag="qk")
            for t in range(9):
                ti, tj = t // 3, t % 3
                base = (r + ti) * WW + c0 + tj
                nc.tensor.matmul(pqk[:, :], xt[:, base:base + 128],
                                 w9qk[:, 256 * t:256 * t + 256],
                                 start=(t == 0), stop=(t == 8))
            rt = rpool.tile([128, 256], BF16, tag="ring")
            # q scaled by sa (per-partition in transposed layout), k plain
            nc.scalar.activation(rt[:, 0:128], pqk[:, 0:128], AF.Copy,
                                 scale=saT[:, j:j + 1])
            nc.vector.tensor_copy(rt[:, 128:256], pqk[:, 128:256])
            nc.tensor.matmul(G[:, 0:256], rt[:, 0:128], rt[:, 0:256],
                             start=(j == 0), stop=(j == NCH_T - 1),
                             skip_group_check=True)
            nc.tensor.matmul(G2[:, :], rt[:, 128:256], rt[:, 128:256],
                             start=(j == 0), stop=(j == NCH_T - 1),
                             skip_group_check=True)

        # ---- v conv (layout A) with mean accumulation ----
        vt = vpool.tile([128, NLOC], BF16, tag="v")
        vts.append(vt)
        vcols = mpool.tile([128, 16], FP32, tag="vcols")
        if not en_v:
            nc.vector.memset(vt[:], 0.0)
            nc.vector.memset(vcols[:], 0.0)
        for nn in range(NCH_A if en_v else 0):
            r0 = 2 * nn
            pv = psA.tile([128, 512], FP32, tag="a")
            for t in range(9):
                ti, tj = t // 3, t % 3
                xv = xt[:].rearrange("p (h w) -> p h w", h=HH)[
                    :, r0 + ti:r0 + ti + 2, tj:tj + 256]
                nc.tensor.matmul(pv[:, :], w9v[:, 128 * t:128 * t + 128], xv,
                                 start=(t == 0), stop=(t == 8))
            nc.scalar.activation(vt[:, 512 * nn:512 * nn + 512], pv[:, :],
                                 AF.Copy, accum_out=vcols[:, nn:nn + 1])

        for rep in range(EXTRA_V):
            for nn in range(NCH_A):
                r0 = 2 * nn
                pv = psA.tile([128, 512], FP32, tag="a")
                for t in range(9):
                    ti, tj = t // 3, t % 3
                    xv = xt[:].rearrange("p (h w) -> p h w", h=HH)[
                        :, r0 + ti:r0 + ti + 2, tj:tj + 256]
                    nc.tensor.matmul(pv[:, :], w9v[:, 128 * t:128 * t + 128], xv,
                                     start=(t == 0), stop=(t == 8))
                nc.scalar.activation(vt[:, 512 * nn:512 * nn + 512], pv[:, :],
                                     AF.Copy, accum_out=vcols[:, nn:nn + 1])

        # ---- stats staging for the AllReduce ----
        junk = mpool.tile([128, 128], FP32, tag="junk")
        nc.vector.tensor_copy(arst[:, cb:cb + 128], G[:, 128:256])  # Gqk
        nc.vector.scalar_tensor_tensor(junk[:], G[:, 0:128], 1.0, eye,
                                       ALU.mult, ALU.mult,
                                       accum_out=arst[:, cb + 128:cb + 129])
        nc.vector.scalar_tensor_tensor(junk[:], G2[:, :], 1.0, eye,
                                       ALU.mult, ALU.mult,
                                       accum_out=arst[:, cb + 129:cb + 130])
        nc.vector.tensor_reduce(arst[:, cb + 130:cb + 131], vcols[:],
                                mybir.AxisListType.X, ALU.add)

    if STAGE < 2:
        # dummy output so the NEFF is valid
        for b in range(min(B, EN_NB)):
            out2d = io["out"][b].rearrange("c h w -> c (h w)")
            for nn in range(NCH_A):
                ot = opool.tile([128, 512], FP32, tag="ot")
                nc.vector.tensor_copy(ot[:], vts[b][:, 512 * nn:512 * nn + 512])
                dma(out2d[:, 512 * nn:512 * nn + 512], ot[:])
        ctx.close()
        return

    # ================= AllReduce of stats =================
    din = dpool.tile([128, 262], FP32, tag="din")
    dout = dpool.tile([128, 262], FP32, tag="dout")
    nc.gpsimd.dma_start(out=din[:], in_=arst[:])
    nc.gpsimd.collective_compute(
        "AllReduce", ALU.add,
        replica_groups=[list(range(NCORES))],
        ins=[din[:].opt()], outs=[dout[:].opt()])
    arres = mpool.tile([128, 262], FP32, tag="arres")
    nc.gpsimd.dma_start(out=arres[:], in_=dout[:])

    if STAGE < 3:
        for b in range(B):
            out2d = io["out"][b].rearrange("c h w -> c (h w)")
            ot = opool.tile([128, 512], FP32, tag="ot")
            nc.vector.tensor_copy(ot[:, 0:262], arres[:])
            nc.vector.tensor_copy(ot[:, 262:512], vts[b][:, 262:512])
            dma(out2d[:, 0:512], ot[:])
        ctx.close()
        return

    # ================= y depthwise conv (overlaps AllReduce) =================
    for b in range(B):
        yt = yts[b]
        y2t = y2pool.tile([128, NLOC], BF16, tag="y2")
        y2ts.append(y2t)
        for nn in range(NCH_A):
            r0 = 2 * nn
            py = psA.tile([128, 512], FP32, tag="a")
            for t in range(9):
                ti, tj = t // 3, t % 3
                yv = yt[:].rearrange("p (h w) -> p h w", h=HH)[
                    :, r0 + ti:r0 + ti + 2, tj:tj + 256]
                nc.tensor.matmul(py[:, :], wyd[:, 128 * t:128 * t + 128], yv,
                                 start=(t == 0), stop=(t == 8))
            if nn % 2 == 0:
                nc.vector.tensor_copy(y2t[:, 512 * nn:512 * nn + 512], py[:, :])
            else:
                nc.scalar.copy(y2t[:, 512 * nn:512 * nn + 512], py[:, :])

    if STAGE < 4:
        for b in range(B):
            out2d = io["out"][b].rearrange("c h w -> c (h w)")
            for nn in range(NCH_A):
                ot = opool.tile([128, 512], FP32, tag="ot")
                nc.vector.tensor_copy(ot[:], y2ts[b][:, 512 * nn:512 * nn + 512])
                dma(out2d[:, 512 * nn:512 * nn + 512], ot[:])
        ctx.close()
        return

    # ================= post-AllReduce attention math =================
    meffts, p2ts = [], []
    for b in range(B):
        cb = 131 * b
        # 1/max(sqrt(d), eps) with one Newton-rsqrt refinement
        rqk = mpool.tile([128, 2], FP32, tag="rqk")
        srt = mpool.tile([128, 2], FP32, tag="srt")
        dcat = arres[:, cb + 128:cb + 130]  # [qd kd]
        nc.scalar.activation(srt[:], dcat, AF.Sqrt)
        nc.vector.tensor_scalar_max(srt[:], srt[:], 1e-12)
        nc.vector.reciprocal(rqk[:], srt[:])
        r2 = mpool.tile([128, 2], FP32, tag="r2")
        nc.vector.tensor_tensor(r2[:], rqk[:], rqk[:], ALU.mult)
        nc.vector.tensor_tensor(r2[:], r2[:], dcat, ALU.mult)
        nc.vector.tensor_scalar(r2[:], r2[:], -0.5, 1.5, ALU.mult, ALU.add)
        nc.vector.tensor_tensor(rqk[:], rqk[:], r2[:], ALU.mult)
        rqt = mpool.tile([128, 1], FP32, tag="rqt")
        nc.vector.tensor_tensor(rqt[:], rqk[:, 0:1], tempp, ALU.mult)

        # broadcast rk along partitions: rkb[p, d] = rk[d]
        ps1 = psA.tile([128, 128], FP32, tag="a")
        nc.tensor.matmul(ps1[0:1, :], rqk[:, 1:2], eye, start=True, stop=True)
        rkrow = mpool.tile([1, 128], FP32, tag="rkrow")
        nc.scalar.copy(rkrow[:], ps1[0:1, :])
        ps2 = psA.tile([128, 128], FP32, tag="a")
        nc.tensor.matmul(ps2[:, :], onesrow, rkrow[:], start=True, stop=True)

        # logits -> masked softmax -> attnBD
        gh = mpool.tile([128, 128], FP32, tag="gh")
        nc.vector.scalar_tensor_tensor(gh[:], arres[:, cb:cb + 128], rqt[:, 0:1],
                                       ps2[:, :], ALU.mult, ALU.mult)
        sm = mpool.tile([128, 128], FP32, tag="sm")
        nc.scalar.activation(sm[:], gh[:], AF.Exp)
        rs = mpool.tile([128, 1], FP32, tag="rs")
        nc.vector.scalar_tensor_tensor(sm[:], sm[:], 1.0, bdmask,
                                       ALU.mult, ALU.mult, accum_out=rs[:])
        nc.vector.reciprocal(rs[:], rs[:])
        attn = mpool.tile([128, 128], FP32, tag="attn")
        nc.vector.tensor_scalar_mul(attn[:], sm[:], rs[:, 0:1])

        # MeffT = (proj @ attnBD)^T = attnBD^T-free matmul: lhsT=attn, rhs=projT
        psM = psA.tile([128, 128], FP32, tag="a")
        nc.tensor.matmul(psM[:, :], attn[:], projt[:], start=True, stop=True)
        mefft = mpool.tile([128, 128], BF16, tag="mefft")
        nc.scalar.copy(mefft[:], psM[:, :])
        meffts.append(mefft)

        # pooled = attnBD @ v_mean  (via attn^T)
        psT = psA.tile([128, 128], FP32, tag="a")
        nc.tensor.transpose(psT[:, :], attn[:], eye)
        attnt = mpool.tile([128, 128], FP32, tag="attnt")
        nc.vector.tensor_copy(attnt[:], psT[:, :])
        psP = psA.tile([128, 1], FP32, tag="a")
        nc.tensor.matmul(psP[:, :], attnt[:], arres[:, cb + 130:cb + 131],
                         start=True, stop=True)
        pooled = mpool.tile([128, 1], FP32, tag="pooled")
        nc.scalar.activation(pooled[:], psP[:, :], AF.Copy, scale=1.0 / NTOT)

        # spectral gate MLP: sigmoid(w3 gelu(w2 gelu(w1 pooled)))
        psg1 = psA.tile([16, 1], FP32, tag="a")
        nc.tensor.matmul(psg1[:, :], spw1t[:], pooled[:], start=True, stop=True)
        g1 = mpool.tile([16, 1], FP32, tag="g1")
        nc.scalar.activation(g1[:], psg1[:, :], AF.Gelu)
        psg2 = psA.tile([16, 1], FP32, tag="a")
        nc.tensor.matmul(psg2[:, :], spw2t[:], g1[:], start=True, stop=True)
        g2 = mpool.tile([16, 1], FP32, tag="g2")
        nc.scalar.activation(g2[:], psg2[:, :], AF.Gelu)
        psg3 = psA.tile([128, 1], FP32, tag="a")
        nc.tensor.matmul(psg3[:, :], spw3t[:], g2[:], start=True, stop=True)
        spec = mpool.tile([128, 1], FP32, tag="spec")
        nc.scalar.activation(spec[:], psg3[:, :], AF.Sigmoid)

        # fold spectral gate into the projection of the dwconv(y) branch
        p2t = mpool.tile([128, 128], BF16, tag="p2t")
        nc.vector.tensor_scalar_mul(p2t[:], projt[:], spec[:, 0:1])
        p2ts.append(p2t)

    if STAGE < 5:
        for b in range(B):
            out2d = io["out"][b].rearrange("c h w -> c (h w)")
            ot = opool.tile([128, 512], FP32, tag="ot")
            nc.vector.tensor_copy(ot[:, 0:128], meffts[b][:])
            nc.vector.tensor_copy(ot[:, 128:256], p2ts[b][:])
            nc.vector.tensor_copy(ot[:, 256:512], vts[b][:, 256:512])
            dma(out2d[:, 0:512], ot[:])
        ctx.close()
        return

    # ================= final fused projection =================
    for b in range(B):
        out2d = io["out"][b].rearrange("c h w -> c (h w)")
        for nn in range(NCH_A):
            pf = psA.tile([128, 512], FP32, tag="a")
            nc.tensor.matmul(pf[:, :], meffts[b][:],
                             vts[b][:, 512 * nn:512 * nn + 512],
                             start=True, stop=False)
            nc.tensor.matmul(pf[:, :], p2ts[b][:],
                             y2ts[b][:, 512 * nn:512 * nn + 512],
                             start=False, stop=True)
            ot = opool.tile([128, 512], FP32, tag="ot")
            nc.scalar.copy(ot[:], pf[:, :])
            dma(out2d[:, 512 * nn:512 * nn + 512], ot[:])

    ctx.close()


def build_nc():
    nc = bacc.Bacc("TRN2", target_bir_lowering=False, debug=False,
                   num_devices=NCORES)
    io = {}

    def inp(name, shape, dt):
        io[name] = nc.dram_tensor(name, shape, dt, kind="ExternalInput")

    inp("xh", [B, C, HH, WW], BF16)
    inp("yh", [B, C, HH, WW], BF16)
    inp("w9qk", [128, 9 * 256], BF16)
    inp("w9v", [128, 9 * 128], BF16)
    inp("wyd", [128, 9 * 128], BF16)
    inp("saw1t", [128, 32], BF16)
    inp("w2rep", [128, 32], BF16)
    inp("w3rep", [128, 1], BF16)
    inp("spw1t", [128, 16], FP32)
    inp("spw2t", [16, 16], FP32)
    inp("spw3t", [16, 128], FP32)
    inp("projt", [128, 128], FP32)
    inp("consts", [128, 386], FP32)
    io["out"] = nc.dram_tensor("out", [B, C, RPC, W], FP32, kind="ExternalOutput")

    with tile.TileContext(nc) as tc:
        _emit(tc, io)
    nc.finalize()
    return nc


_CACHE = {}


def _prep_host(x, y, qkv_w, qkv_dw_w, proj_w, sa_w1, sa_w2, sa_w3,
               sp_w1, sp_w2, sp_w3, dw_w, temperature):
    import ml_dtypes
    bf = ml_dtypes.bfloat16
    f32 = np.float32

    x = np.asarray(x, f32)
    y = np.asarray(y, f32)
    xp = np.zeros((B, C, H + 2, W + 2), f32)
    xp[:, :, 1:H + 1, 1:W + 1] = x
    yp = np.zeros((B, C, H + 2, W + 2), f32)
    yp[:, :, 1:H + 1, 1:W + 1] = y
    xp = xp.astype(bf)
    yp = yp.astype(bf)

    qkv_w = np.asarray(qkv_w, f32)
    dw = np.asarray(qkv_dw_w, f32).reshape(3 * C, 9)
    w9qk = np.concatenate(
        [(qkv_w[:256] * dw[:256, t:t + 1]).T for t in range(9)], axis=1)  # [128, 9*256]
    w9v = np.concatenate(
        [(qkv_w[256:] * dw[256:, t:t + 1]).T for t in range(9)], axis=1)  # [128, 9*128]
    dwy = np.asarray(dw_w, f32).reshape(C, 9)
    wyd = np.concatenate(
        [np.diag(dwy[:, t]) for t in range(9)], axis=1)                  # [128, 9*128]

    w2rep = np.zeros((128, 32), f32)
    w3rep = np.zeros((128, 1), f32)
    for k in range(4):
        w2rep[32 * k:32 * k + 16, 0:16] = np.asarray(sa_w2, f32).T
        w3rep[32 * k:32 * k + 16] = np.asarray(sa_w3, f32).T
    saw1tp = np.zeros((128, 32), f32)
    saw1tp[:, 0:16] = np.asarray(sa_w1, f32).T

    consts = np.zeros((128, 386), f32)
    consts[:, 0:128] = np.eye(128, dtype=f32)
    ci = np.arange(128) // DH
    consts[:, 128:256] = (ci[:, None] == ci[None, :]).astype(f32)
    consts[:, 256] = np.asarray(temperature, f32).reshape(HD)[ci]
    consts[0, 257:385] = 1.0

    common = {
        "w9qk": w9qk.astype(bf), "w9v": w9v.astype(bf), "wyd": wyd.astype(bf),
        "saw1t": saw1tp.astype(bf),
        "w2rep": w2rep.astype(bf), "w3rep": w3rep.astype(bf),
        "spw1t": np.asarray(sp_w1, f32).T.copy(),
        "spw2t": np.asarray(sp_w2, f32).T.copy(),
        "spw3t": np.asarray(sp_w3, f32).T.copy(),
        "projt": np.asarray(proj_w, f32).T.copy(),
        "consts": consts,
    }
    in_maps = []
    for i in range(NCORES):
        m = dict(common)
        m["xh"] = np.ascontiguousarray(xp[:, :, 32 * i:32 * i + HH, :])
        m["yh"] = np.ascontiguousarray(yp[:, :, 32 * i:32 * i + HH, :])
        in_maps.append(m)
    return in_maps


def kernel(**inputs):
    if "nc" not in _CACHE:
        _CACHE["nc"] = build_nc()
    nc = _CACHE["nc"]
    in_maps = _prep_host(**inputs)
    res = run_bass_kernel_spmd(nc, in_maps, core_ids=list(range(NCORES)))
    shards = [res.results[i]["out"] for i in range(NCORES)]
    return np.concatenate(shards, axis=2).astype(np.float32)



# revision 4
# speedup vs baseline: 1.0432x; 1.0432x over previous
"""Cross-Spatial-Attention Trainium2 kernel (8 NeuronCores, spatial sharding).

Strategy: shard the 256-row image into 8 bands of 32 rows (both batch elements
on every core, 1-row halos for the 3x3 depthwise convs). All convolutions and
the attention application are then fully local; the only cross-core data is the
channel-gram / norm / mean statistics (two small AllReduces).

Key formulations:
  - dwconv3x3(conv1x1(x)) == sum over 9 taps of shifted matmuls with
    per-tap-combined weights (PSUM accumulation) for the qk and v branches.
  - q,k are produced directly transposed ([n,c] layout) via
    out_chunk = x_chunk^T @ W_tap^T, so the channel gram needs no transpose
    pass and the spatial gate `sa` is a per-partition scalar.
  - the y-branch depthwise conv runs on the Vector engine (9 shifted
    multiply-accumulates in bf16 2x mode; odd-column taps read a host-shifted
    copy of y to keep 4-byte alignment), freeing ~75us of PE time.
  - stats AllReduce is split: gram/norms reduce right after the qk convs and
    is hidden by the v convs; the tiny v-mean reduce follows and is hidden by
    the softmax/Meff math.
  - softmax over a full 128x128 gram with a block-diagonal mask; the
    attention apply + output projection collapse into one matmul
    (Meff = proj @ attnBD), and the spectral gate folds into the
    projection weights for the dwconv(y) branch.
"""

import numpy as np
from contextlib import ExitStack

import concourse.bass as bass
import concourse.bacc as bacc
import concourse.tile as tile
from concourse import mybir
from concourse.bass_utils import run_bass_kernel_spmd

FP32 = mybir.dt.float32
BF16 = mybir.dt.bfloat16
AF = mybir.ActivationFunctionType
ALU = mybir.AluOpType

B, C, H, W = 2, 128, 256, 256
HD, DH = 8, 16
NCORES = 8
RPC = H // NCORES            # 32 rows per core
HH, WW = RPC + 2, W + 2      # 34 x 258 halo'd band
FREE = HH * WW               # 8772
NLOC = RPC * W               # 8192 output positions per band per batch
NCH_T = NLOC // 128          # 64 transposed chunks
NCH_A = NLOC // 512          # 16 layout-A chunks
NTOT = float(H * W)          # global spatial size


def _emit(tc, io):
    nc = tc.nc
    ctx = ExitStack()

    wpool = ctx.enter_context(tc.tile_pool(name="wpool", bufs=1))
    xpool = ctx.enter_context(tc.tile_pool(name="xpool", bufs=2))
    ypool = ctx.enter_context(tc.tile_pool(name="ypool", bufs=2))
    yspool = ctx.enter_context(tc.tile_pool(name="yspool", bufs=2))
    vpool = ctx.enter_context(tc.tile_pool(name="vpool", bufs=2))
    y2pool = ctx.enter_context(tc.tile_pool(name="y2pool", bufs=2))
    spool = ctx.enter_context(tc.tile_pool(name="spool", bufs=1))
    rpool = ctx.enter_context(tc.tile_pool(name="rpool", bufs=6))
    mpool = ctx.enter_context(tc.tile_pool(name="mpool", bufs=2))
    opool = ctx.enter_context(tc.tile_pool(name="opool", bufs=3))
    psA = ctx.enter_context(tc.tile_pool(name="psA", bufs=3, space="PSUM"))
    psQK = ctx.enter_context(tc.tile_pool(name="psQK", bufs=3, space="PSUM"))
    psG = ctx.enter_context(tc.tile_pool(name="psG", bufs=1, space="PSUM"))
    dpool = ctx.enter_context(tc.tile_pool(name="dram", bufs=1, space="DRAM"))

    # ---- load weights/constants (sync queue) ----
    def wload(name, shape, dt=BF16):
        t = wpool.tile(shape, dt, tag=name)
        nc.sync.dma_start(out=t[:], in_=io[name][:])
        return t

    w9qk = wload("w9qk", [128, 9 * 256])     # tap t at cols [256t:256t+256]
    w9v = wload("w9v", [128, 9 * 128])
    saw1t = wload("saw1t", [128, 32])
    w2rep = wload("w2rep", [128, 32])
    w3rep = wload("w3rep", [128, 1])
    spw1t = wload("spw1t", [128, 16], FP32)
    spw2t = wload("spw2t", [16, 16], FP32)
    spw3t = wload("spw3t", [16, 128], FP32)
    projt = wload("projt", [128, 128], FP32)
    wydc = wload("wydc", [128, 9], FP32)
    consts = wload("consts", [128, 386], FP32)
    eye = consts[:, 0:128]
    bdmask = consts[:, 128:256]
    tempp = consts[:, 256:257]
    onesrow = consts[0:1, 257:385]

    # ---- input DMAs: y first (chunked, sync), x + yshift on other queues ----
    xts, yts, yshs, vts, y2ts, saTs = [], [], [], [], [], []
    SPLIT = 17 * WW
    for b in range(B):
        yt = ypool.tile([128, FREE], BF16, tag="y")
        y2d = io["yh"][b].rearrange("c h w -> c (h w)")
        nc.sync.dma_start(out=yt[:, 0:SPLIT], in_=y2d[:, 0:SPLIT])
        nc.sync.dma_start(out=yt[:, SPLIT:FREE], in_=y2d[:, SPLIT:FREE])
        yts.append(yt)
    for b in range(B):
        xt = xpool.tile([128, FREE], BF16, tag="x")
        eng = nc.gpsimd if b == 0 else nc.scalar
        eng.dma_start(out=xt[:], in_=io["xh"][b].rearrange("c h w -> c (h w)"))
        xts.append(xt)
    for b in range(B):
        ysh = yspool.tile([128, FREE], BF16, tag="ysh")
        eng = nc.gpsimd if b == 0 else nc.scalar
        eng.dma_start(out=ysh[:],
                      in_=io["ysh"][b].rearrange("c h w -> c (h w)"))
        yshs.append(ysh)

    arst1 = mpool.tile([128, 260], FP32, tag="arst1")
    arst2 = mpool.tile([128, 2], FP32, tag="arst2")

    # ---- y depthwise conv steps (vector engine, interleaved into qk/v) ----
    ydw_steps = []

    def make_ydw(b):
        yt, ysh = yts[b], yshs[b]
        y2t = y2pool.tile([128, NLOC], BF16, tag="y2")
        y2ts.append(y2t)
        for nn in range(NCH_A):
            r0 = 2 * nn
            ov = y2t[:, 512 * nn:512 * nn + 512].rearrange(
                "p (r w) -> p r w", r=2)
            for t in range(9):
                ti, tj = t // 3, t % 3
                src, tjj = (ysh, 1) if tj == 1 else (yt, tj)
                if tj == 1:
                    tjj = 0
                iv = src[:].rearrange("p (h w) -> p h w", h=HH)[
                    :, r0 + ti:r0 + ti + 2, tjj:tjj + 256]
                wc = wydc[:, t:t + 1]
                if t == 0:
                    ydw_steps.append(
                        lambda ov=ov, iv=iv, wc=wc:
                        nc.vector.tensor_scalar_mul(ov, iv, wc))
                else:
                    ydw_steps.append(
                        lambda ov=ov, iv=iv, wc=wc:
                        nc.vector.scalar_tensor_tensor(
                            ov, iv, wc, ov, ALU.mult, ALU.add))

    def pop_ydw(n):
        for _ in range(n):
            if ydw_steps:
                ydw_steps.pop(0)()

    # ================= per-batch: sa gate + qk conv/gram =================
    for b in range(B):
        cb = 130 * b
        xt, yt = xts[b], yts[b]
        make_ydw(b)

        # ---- spatial-attention gate: sa = sigmoid(w3 relu(w2 relu(w1 y))) ----
        s1 = spool.tile([128, 2048], BF16, tag="s1")
        s2 = spool.tile([128, 2048], BF16, tag="s2")
        for g in range(4):
            ps1 = psA.tile([128, 512], FP32, tag="a")
            for k in range(4):
                nn = 4 * g + k
                r0 = 2 * nn
                yv = yt[:].rearrange("p (h w) -> p h w", h=HH)[
                    :, r0 + 1:r0 + 3, 1:257]
                nc.tensor.matmul(ps1[32 * k:32 * k + 32, :], saw1t[:, :], yv,
                                 start=True, stop=True, tile_position=(0, 32 * k))
            if g % 2 == 0:
                nc.vector.tensor_scalar_max(s1[:, 512 * g:512 * g + 512], ps1[:, :], 0.0)
            else:
                nc.scalar.activation(s1[:, 512 * g:512 * g + 512], ps1[:, :], AF.Relu)
        for g in range(4):
            ps2 = psA.tile([128, 512], FP32, tag="a")
            for k in range(4):
                nc.tensor.matmul(ps2[32 * k:32 * k + 32, :],
                                 w2rep[32 * k:32 * k + 16, :],
                                 s1[32 * k:32 * k + 16, 512 * g:512 * g + 512],
                                 start=True, stop=True,
                                 tile_position=(32 * k, 32 * k))
            if g % 2 == 0:
                nc.vector.tensor_scalar_max(s2[:, 512 * g:512 * g + 512], ps2[:, :], 0.0)
            else:
                nc.scalar.activation(s2[:, 512 * g:512 * g + 512], ps2[:, :], AF.Relu)
        # stage 3: saT[n] packed as [128, 64] (col j holds n = 128j + p)
        saT_ps = psQK.tile([128, 64], FP32, tag="qk")
        for j in range(NCH_T):
            nn, off = j // 4, (j % 4) * 128
            g, k = nn // 4, nn % 4
            nc.tensor.matmul(saT_ps[:, j:j + 1],
                             s2[32 * k:32 * k + 16,
                                512 * g + off:512 * g + off + 128],
                             w3rep[32 * k:32 * k + 16, :],
                             start=True, stop=True, tile_position=(32 * k, 0))
        saT = mpool.tile([128, 64], FP32, tag="saT")
        nc.scalar.activation(saT[:], saT_ps[:], AF.Sigmoid)
        saTs.append(saT)

        # ---- qk conv (transposed layout) + gram accumulation ----
        G = psG.tile([128, 256], FP32, tag="G")
        G2 = psG.tile([128, 128], FP32, tag="G2")
        for j in range(NCH_T):
            r, c0 = j // 2, (j % 2) * 128
            pqk = psQK.tile([128, 256], FP32, tag="qk")
            for t in range(9):
                ti, tj = t // 3, t % 3
                base = (r + ti) * WW + c0 + tj
                nc.tensor.matmul(pqk[:, :], xt[:, base:base + 128],
                                 w9qk[:, 256 * t:256 * t + 256],
                                 start=(t == 0), stop=(t == 8))
            rt = rpool.tile([128, 256], BF16, tag="ring")
            # q scaled by sa (per-partition in transposed layout), k plain
            nc.scalar.activation(rt[:, 0:128], pqk[:, 0:128], AF.Copy,
                                 scale=saT[:, j:j + 1])
            nc.vector.tensor_copy(rt[:, 128:256], pqk[:, 128:256])
            nc.tensor.matmul(G[:, 0:256], rt[:, 0:128], rt[:, 0:256],
                             start=(j == 0), stop=(j == NCH_T - 1),
                             skip_group_check=True)
            nc.tensor.matmul(G2[:, :], rt[:, 128:256], rt[:, 128:256],
                             start=(j == 0), stop=(j == NCH_T - 1),
                             skip_group_check=True)
            pop_ydw(2)

        # ---- stats staging for AllReduce 1 ----
        junk = mpool.tile([128, 128], FP32, tag="junk")
        nc.vector.tensor_copy(arst1[:, cb:cb + 128], G[:, 128:256])  # Gqk
        nc.vector.scalar_tensor_tensor(junk[:], G[:, 0:128], 1.0, eye,
                                       ALU.mult, ALU.mult,
                                       accum_out=arst1[:, cb + 128:cb + 129])
        nc.vector.scalar_tensor_tensor(junk[:], G2[:, :], 1.0, eye,
                                       ALU.mult, ALU.mult,
                                       accum_out=arst1[:, cb + 129:cb + 130])

    # ================= AllReduce 1: gram + norms =================
    din1 = dpool.tile([128, 260], FP32, tag="din1")
    dout1 = dpool.tile([128, 260], FP32, tag="dout1")
    nc.gpsimd.dma_start(out=din1[:], in_=arst1[:])
    nc.gpsimd.collective_compute(
        "AllReduce", ALU.add,
        replica_groups=[list(range(NCORES))],
        ins=[din1[:].opt()], outs=[dout1[:].opt()])
    arres1 = mpool.tile([128, 260], FP32, tag="arres1")
    nc.gpsimd.dma_start(out=arres1[:], in_=dout1[:])

    # ================= v convs (hide AllReduce 1) =================
    for b in range(B):
        xt = xts[b]
        vt = vpool.tile([128, NLOC], BF16, tag="v")
        vts.append(vt)
        vcols = mpool.tile([128, 16], FP32, tag="vcols")
        for nn in range(NCH_A):
            r0 = 2 * nn
            pv = psA.tile([128, 512], FP32, tag="a")
            for t in range(9):
                ti, tj = t // 3, t % 3
                xv = xt[:].rearrange("p (h w) -> p h w", h=HH)[
                    :, r0 + ti:r0 + ti + 2, tj:tj + 256]
                nc.tensor.matmul(pv[:, :], w9v[:, 128 * t:128 * t + 128], xv,
                                 start=(t == 0), stop=(t == 8))
            nc.scalar.activation(vt[:, 512 * nn:512 * nn + 512], pv[:, :],
                                 AF.Copy, accum_out=vcols[:, nn:nn + 1])
            pop_ydw(1)
        nc.vector.tensor_reduce(arst2[:, b:b + 1], vcols[:],
                                mybir.AxisListType.X, ALU.add)
    pop_ydw(len(ydw_steps))

    # ================= AllReduce 2: v means =================
    din2 = dpool.tile([128, 2], FP32, tag="din2")
    dout2 = dpool.tile([128, 2], FP32, tag="dout2")
    nc.gpsimd.dma_start(out=din2[:], in_=arst2[:])
    nc.gpsimd.collective_compute(
        "AllReduce", ALU.add,
        replica_groups=[list(range(NCORES))],
        ins=[din2[:].opt()], outs=[dout2[:].opt()])
    arres2 = mpool.tile([128, 2], FP32, tag="arres2")
    nc.gpsimd.dma_start(out=arres2[:], in_=dout2[:])

    # ================= post-AllReduce-1 attention math =================
    meffts, attnts = [], []
    for b in range(B):
        cb = 130 * b
        # 1/max(sqrt(d), eps) with one Newton-rsqrt refinement
        rqk = mpool.tile([128, 2], FP32, tag="rqk")
        srt = mpool.tile([128, 2], FP32, tag="srt")
        dcat = arres1[:, cb + 128:cb + 130]  # [qd kd]
        nc.scalar.activation(srt[:], dcat, AF.Sqrt)
        nc.vector.tensor_scalar_max(srt[:], srt[:], 1e-12)
        nc.vector.reciprocal(rqk[:], srt[:])
        r2 = mpool.tile([128, 2], FP32, tag="r2")
        nc.vector.tensor_tensor(r2[:], rqk[:], rqk[:], ALU.mult)
        nc.vector.tensor_tensor(r2[:], r2[:], dcat, ALU.mult)
        nc.vector.tensor_scalar(r2[:], r2[:], -0.5, 1.5, ALU.mult, ALU.add)
        nc.vector.tensor_tensor(rqk[:], rqk[:], r2[:], ALU.mult)
        rqt = mpool.tile([128, 1], FP32, tag="rqt")
        nc.vector.tensor_tensor(rqt[:], rqk[:, 0:1], tempp, ALU.mult)

        # broadcast rk along partitions: rkb[p, d] = rk[d]
        ps1 = psA.tile([128, 128], FP32, tag="a")
        nc.tensor.matmul(ps1[0:1, :], rqk[:, 1:2], eye, start=True, stop=True)
        rkrow = mpool.tile([1, 128], FP32, tag="rkrow")
        nc.scalar.copy(rkrow[:], ps1[0:1, :])
        ps2 = psA.tile([128, 128], FP32, tag="a")
        nc.tensor.matmul(ps2[:, :], onesrow, rkrow[:], start=True, stop=True)

        # logits -> masked softmax -> attnBD
        gh = mpool.tile([128, 128], FP32, tag="gh")
        nc.vector.scalar_tensor_tensor(gh[:], arres1[:, cb:cb + 128], rqt[:, 0:1],
                                       ps2[:, :], ALU.mult, ALU.mult)
        sm = mpool.tile([128, 128], FP32, tag="sm")
        nc.scalar.activation(sm[:], gh[:], AF.Exp)
        rs = mpool.tile([128, 1], FP32, tag="rs")
        nc.vector.scalar_tensor_tensor(sm[:], sm[:], 1.0, bdmask,
                                       ALU.mult, ALU.mult, accum_out=rs[:])
        nc.vector.reciprocal(rs[:], rs[:])
        attn = mpool.tile([128, 128], FP32, tag="attn")
        nc.vector.tensor_scalar_mul(attn[:], sm[:], rs[:, 0:1])

        # MeffT = (proj @ attnBD)^T: lhsT=attn, rhs=projT
        psM = psA.tile([128, 128], FP32, tag="a")
        nc.tensor.matmul(psM[:, :], attn[:], projt[:], start=True, stop=True)
        mefft = mpool.tile([128, 128], BF16, tag="mefft")
        nc.scalar.copy(mefft[:], psM[:, :])
        meffts.append(mefft)

        # attn^T (for pooled = attnBD @ v_mean)
        psT = psA.tile([128, 128], FP32, tag="a")
        nc.tensor.transpose(psT[:, :], attn[:], eye)
        attnt = mpool.tile([128, 128], FP32, tag="attnt")
        nc.vector.tensor_copy(attnt[:], psT[:, :])
        attnts.append(attnt)

    # ============ post-AllReduce-2 spectral gate + final projection ============
    for b in range(B):
        psP = psA.tile([128, 1], FP32, tag="a")
        nc.tensor.matmul(psP[:, :], attnts[b][:], arres2[:, b:b + 1],
                         start=True, stop=True)
        pooled = mpool.tile([128, 1], FP32, tag="pooled")
        nc.scalar.activation(pooled[:], psP[:, :], AF.Copy, scale=1.0 / NTOT)

        # spectral gate MLP: sigmoid(w3 gelu(w2 gelu(w1 pooled)))
        psg1 = psA.tile([16, 1], FP32, tag="a")
        nc.tensor.matmul(psg1[:, :], spw1t[:], pooled[:], start=True, stop=True)
        g1 = mpool.tile([16, 1], FP32, tag="g1")
        nc.scalar.activation(g1[:], psg1[:, :], AF.Gelu)
        psg2 = psA.tile([16, 1], FP32, tag="a")
        nc.tensor.matmul(psg2[:, :], spw2t[:], g1[:], start=True, stop=True)
        g2 = mpool.tile([16, 1], FP32, tag="g2")
        nc.scalar.activation(g2[:], psg2[:, :], AF.Gelu)
        psg3 = psA.tile([128, 1], FP32, tag="a")
        nc.tensor.matmul(psg3[:, :], spw3t[:], g2[:], start=True, stop=True)
        spec = mpool.tile([128, 1], FP32, tag="spec")
        nc.scalar.activation(spec[:], psg3[:, :], AF.Sigmoid)

        # fold spectral gate into the projection of the dwconv(y) branch
        p2t = mpool.tile([128, 128], BF16, tag="p2t")
        nc.vector.tensor_scalar_mul(p2t[:], projt[:], spec[:, 0:1])

        # final fused projection
        out2d = io["out"][b].rearrange("c h w -> c (h w)")
        for nn in range(NCH_A):
            pf = psA.tile([128, 512], FP32, tag="a")
            nc.tensor.matmul(pf[:, :], meffts[b][:],
                             vts[b][:, 512 * nn:512 * nn + 512],
                             start=True, stop=False)
            nc.tensor.matmul(pf[:, :], p2t[:],
                             y2ts[b][:, 512 * nn:512 * nn + 512],
                             start=False, stop=True)
            ot = opool.tile([128, 512], FP32, tag="ot")
            if nn % 2 == 0:
                nc.scalar.copy(ot[:], pf[:, :])
                nc.sync.dma_start(out=out2d[:, 512 * nn:512 * nn + 512], in_=ot[:])
            else:
                nc.vector.tensor_copy(ot[:], pf[:, :])
                nc.gpsimd.dma_start(out=out2d[:, 512 * nn:512 * nn + 512], in_=ot[:])

    ctx.close()


def build_nc():
    nc = bacc.Bacc("TRN2", target_bir_lowering=False, debug=False,
                   num_devices=NCORES)
    io = {}

    def inp(name, shape, dt):
        io[name] = nc.dram_tensor(name, shape, dt, kind="ExternalInput")

    inp("xh", [B, C, HH, WW], BF16)
    inp("yh", [B, C, HH, WW], BF16)
    inp("ysh", [B, C, HH, WW], BF16)
    inp("w9qk", [128, 9 * 256], BF16)
    inp("w9v", [128, 9 * 128], BF16)
    inp("saw1t", [128, 32], BF16)
    inp("w2rep", [128, 32], BF16)
    inp("w3rep", [128, 1], BF16)
    inp("spw1t", [128, 16], FP32)
    inp("spw2t", [16, 16], FP32)
    inp("spw3t", [16, 128], FP32)
    inp("projt", [128, 128], FP32)
    inp("wydc", [128, 9], FP32)
    inp("consts", [128, 386], FP32)
    io["out"] = nc.dram_tensor("out", [B, C, RPC, W], FP32, kind="ExternalOutput")

    with tile.TileContext(nc) as tc:
        _emit(tc, io)
    nc.finalize()
    return nc


_CACHE = {}


def _prep_host(x, y, qkv_w, qkv_dw_w, proj_w, sa_w1, sa_w2, sa_w3,
               sp_w1, sp_w2, sp_w3, dw_w, temperature):
    import ml_dtypes
    bf = ml_dtypes.bfloat16
    f32 = np.float32

    x = np.asarray(x, f32)
    y = np.asarray(y, f32)
    xp = np.zeros((B, C, H + 2, W + 2), f32)
    xp[:, :, 1:H + 1, 1:W + 1] = x
    yp = np.zeros((B, C, H + 2, W + 2), f32)
    yp[:, :, 1:H + 1, 1:W + 1] = y
    ysp = np.zeros((B, C, H + 2, W + 2), f32)
    ysp[:, :, :, 0:W + 1] = yp[:, :, :, 1:W + 2]   # y shifted left by 1 col
    xp = xp.astype(bf)
    yp = yp.astype(bf)
    ysp = ysp.astype(bf)

    qkv_w = np.asarray(qkv_w, f32)
    dw = np.asarray(qkv_dw_w, f32).reshape(3 * C, 9)
    w9qk = np.concatenate(
        [(qkv_w[:256] * dw[:256, t:t + 1]).T for t in range(9)], axis=1)  # [128, 9*256]
    w9v = np.concatenate(
        [(qkv_w[256:] * dw[256:, t:t + 1]).T for t in range(9)], axis=1)  # [128, 9*128]
    wydc = np.asarray(dw_w, f32).reshape(C, 9)                            # [128, 9]

    w2rep = np.zeros((128, 32), f32)
    w3rep = np.zeros((128, 1), f32)
    for k in range(4):
        w2rep[32 * k:32 * k + 16, 0:16] = np.asarray(sa_w2, f32).T
        w3rep[32 * k:32 * k + 16] = np.asarray(sa_w3, f32).T
    saw1tp = np.zeros((128, 32), f32)
    saw1tp[:, 0:16] = np.asarray(sa_w1, f32).T

    consts = np.zeros((128, 386), f32)
    consts[:, 0:128] = np.eye(128, dtype=f32)
    ci = np.arange(128) // DH
    consts[:, 128:256] = (ci[:, None] == ci[None, :]).astype(f32)
    consts[:, 256] = np.asarray(temperature, f32).reshape(HD)[ci]
    consts[0, 257:385] = 1.0

    common = {
        "w9qk": w9qk.astype(bf), "w9v": w9v.astype(bf),
        "saw1t": saw1tp.astype(bf),
        "w2rep": w2rep.astype(bf), "w3rep": w3rep.astype(bf),
        "spw1t": np.asarray(sp_w1, f32).T.copy(),
        "spw2t": np.asarray(sp_w2, f32).T.copy(),
        "spw3t": np.asarray(sp_w3, f32).T.copy(),
        "projt": np.asarray(proj_w, f32).T.copy(),
        "wydc": wydc,
        "consts": consts,
    }
    in_maps = []
    for i in range(NCORES):
        m = dict(common)
        m["xh"] = np.ascontiguousarray(xp[:, :, 32 * i:32 * i + HH, :])
        m["yh"] = np.ascontiguousarray(yp[:, :, 32 * i:32 * i + HH, :])
        m["ysh"] = np.ascontiguousarray(ysp[:, :, 32 * i:32 * i + HH, :])
        in_maps.append(m)
    return in_maps


def kernel(**inputs):
    if "nc" not in _CACHE:
        _CACHE["nc"] = build_nc()
    nc = _CACHE["nc"]
    in_maps = _prep_host(**inputs)
    res = run_bass_kernel_spmd(nc, in_maps, core_ids=list(range(NCORES)))
    shards = [res.results[i]["out"] for i in range(NCORES)]
    return np.concatenate(shards, axis=2).astype(np.float32)
